# revision 15
# baseline (speedup 1.0000x reference)
"""Trainium2 Bass kernel for nn_CustomClassificationLoss_48765058678812.

Loss (see reference): per sample b with target t, each class c at circular
distance d(c,t) = min((c-t)%360, (t-c)%360) contributes |0.98**d - x[b,c]|
(d=0 gives 1-x, valid since x in [0,1)), except d == 180 contributes 0.
loss = sum over all (b, c) / B.

v2 design (DMA-roofline oriented):
  - Host ships NEGATED logits in f16 (halves HBM traffic vs f32), a constant
    [360, 384] f16 table wtab[t, c] = 0.98**d(c, t) (rows padded to 768 B
    for the gather engine), the targets as a wrapped int16 index tile, and
    (t+180)%360 as per-sample f32 scalars.
  - Per chunk of 1024 samples, one gpsimd dma_gather (mlp ucode library)
    fetches the per-sample weight rows wtab[t] into SBUF; sample s lands on
    partition s%128, slot s//128, and x is DMAed in the matching
    interleaved layout.
  - DVE tensor_add produces dif = W + (-x) = W - x; ACT Abs gives |dif|.
  - Per 128-sample group, one DVE scalar_tensor_tensor computes
    (iota != t180) * |dif| with fused accum -> acc[:, g]. That zeroes the
    d == 180 class exactly and reduces in the same instruction.

Pure data parallel over 8 cores (8192 samples each); host sums the per-core
[128, 64] partials and divides by B.

Notes for the pinned toolchain:
  - clear_and_free_semaphores patch: the pinned walrus rejects the
    EVENT_SEMAPHORE_RANGE_CLEAR ISA blob; keep allocator bookkeeping only.
  - _split_multi_waits: the pinned walrus accepts one sem-wait per
    instruction; hoist extras onto injected NoOps.
  - lower_extended_insts(nc): raw Bass skips codegen_inst_isa_subclasses;
    without it InstISA blobs (the mlp library load) are empty and walrus
    fails with "ISA wrong length".
"""

import numpy as np
from contextlib import ExitStack

import concourse.bass as bass
import concourse.tile as tile
from concourse import mybir
from concourse.bass_utils import run_bass_kernel_spmd
from concourse.library_config import mlp as _mlp_lib
from concourse.library_overlay import lower_extended_insts

NUM_CLASSES = 360
PAD_CLASSES = 384                   # gather rows must be a multiple of 256 B
DECAY = 0.98
N_CORES = 8
B_TOTAL = 65536
B_SHARD = B_TOTAL // N_CORES        # 8192
GROUPS = B_SHARD // 128             # 64 groups of 128 samples
NCHUNK = 8                          # chunks per shard
GPC = GROUPS // NCHUNK              # groups per chunk
SPC = 128 * GPC                     # samples per chunk

_CACHE: dict = {}


def _patched_clear_and_free_semaphores(self, sems):
    if not sems:
        return
    sem_nums = [s.num if hasattr(s, "num") else s for s in sems]
    self._state.prepend_free_semaphores(sem_nums)
    for poison_set in self._tile_sem_poison_stack:
        poison_set.update(sem_nums)


def _split_multi_waits(nc):
    for f in nc.m.functions:
        for b in f.blocks:
            out = []
            changed = False
            for ins in b.instructions:
                si = ins.sync_info
                waits = list(si.on_wait) if (si and si.on_wait) else []
                if len(waits) > 1 and ins.engine is not None:
                    for j, w in enumerate(waits[:-1]):
                        nop = mybir.InstNoOp(
                            name=f"{ins.name}_hw{j}", engine=ins.engine,
                            ins=[], outs=[],
                        )
                        nop.sync_info = mybir.SyncInfo(on_wait=[w], on_update=[])
                        nc.register_instruction(nop)
                        out.append(nop)
                    si.on_wait = [waits[-1]]
                    changed = True
                out.append(ins)
            if changed:
                b.instructions = out


def _build_wtab() -> np.ndarray:
    i = np.arange(NUM_CLASSES)
    delta = (i[None, :] - i[:, None]) % NUM_CLASSES
    dist = np.minimum(delta, NUM_CLASSES - delta)
    w = (DECAY ** dist.astype(np.float64)).astype(np.float16)   # [t, c]
    out = np.zeros((NUM_CLASSES, PAD_CLASSES), np.float16)
    out[:, :NUM_CLASSES] = w
    return out


def _build_nc() -> bass.Bass:
    bass.Bass.clear_and_free_semaphores = _patched_clear_and_free_semaphores
    nc = bass.Bass()
    f16 = mybir.dt.float16
    f32 = mybir.dt.float32
    i16 = mybir.dt.int16

    X = nc.dram_tensor("xf16", [B_SHARD, NUM_CLASSES], f16, kind="ExternalInput")
    IX = nc.dram_tensor("ix", [128, B_SHARD // 16], i16, kind="ExternalInput")
    T180 = nc.dram_tensor("t180", [128, GROUPS], f32, kind="ExternalInput")
    IOT = nc.dram_tensor("iot", [128, NUM_CLASSES], f16, kind="ExternalInput")
    WT = nc.dram_tensor("wtab", [NUM_CLASSES, PAD_CLASSES], f16,
                        kind="ExternalInput")
    OUT = nc.dram_tensor("acc", [128, GROUPS], f32, kind="ExternalOutput")

    # sample s -> partition s%128, group s//128 (dma_gather's layout)
    x_r = X.rearrange("(n p) c -> p n c", p=128)

    with tile.TileContext(nc) as tc, ExitStack() as ctx:
        singles = ctx.enter_context(tc.tile_pool(name="singles", bufs=1))
        xpool = ctx.enter_context(tc.tile_pool(name="xpool", bufs=3))
        wpool = ctx.enter_context(tc.tile_pool(name="wpool", bufs=3))
        dpool = ctx.enter_context(tc.tile_pool(name="dpool", bufs=3))

        # dma_gather (InstDMAGatherAnt) lives in the mlp gpsimd ucode library
        nc.gpsimd.load_library(_mlp_lib)

        ix_sb = singles.tile([128, B_SHARD // 16], i16)
        nc.sync.dma_start(out=ix_sb, in_=IX[:, :])
        t180_sb = singles.tile([128, GROUPS], f32)
        nc.sync.dma_start(out=t180_sb, in_=T180[:, :])
        iot_sb = singles.tile([128, NUM_CLASSES], f16)
        nc.sync.dma_start(out=iot_sb, in_=IOT[:, :])
        acc = singles.tile([128, GROUPS], f32)
        scr = singles.tile([128, NUM_CLASSES], f16)

        for i in range(NCHUNK):
            xt = xpool.tile([128, GPC, NUM_CLASSES], f16, tag="xt")
            nc.sync.dma_start(out=xt, in_=x_r[:, i * GPC:(i + 1) * GPC, :])
            wg = wpool.tile([128, GPC, PAD_CLASSES], f16, tag="wg")
            nc.gpsimd.dma_gather(
                out_ap=wg, in_ap=WT[:, :],
                idxs_ap=ix_sb[:, i * (SPC // 16):(i + 1) * (SPC // 16)],
                num_idxs=SPC, num_idxs_reg=SPC,
                elem_size=PAD_CLASSES,
            )
            dif = dpool.tile([128, GPC, NUM_CLASSES], f16, tag="dif")
            nc.vector.tensor_add(
                out=dif, in0=wg[:, :, 0:NUM_CLASSES], in1=xt
            )
            nc.scalar.activation(
                out=dif, in_=dif, func=mybir.ActivationFunctionType.Abs,
            )
            for g in range(GPC):
                gg = i * GPC + g
                nc.vector.scalar_tensor_tensor(
                    out=scr, in0=iot_sb, scalar=t180_sb[:, gg:gg + 1],
                    in1=dif[:, g, :],
                    op0=mybir.AluOpType.not_equal, op1=mybir.AluOpType.mult,
                    accum_out=acc[:, gg:gg + 1],
                )

        nc.sync.dma_start(out=OUT[:, :], in_=acc)

    lower_extended_insts(nc)
    _split_multi_waits(nc)
    nc.finalize()
    return nc


def _get_nc() -> bass.Bass:
    if "nc" not in _CACHE:
        _CACHE["nc"] = _build_nc()
    return _CACHE["nc"]


def _prep_in_maps(logits: np.ndarray, targets: np.ndarray) -> list[dict]:
    if "wtab" not in _CACHE:
        _CACHE["wtab"] = _build_wtab()
        _CACHE["iot"] = np.ascontiguousarray(
            np.broadcast_to(np.arange(NUM_CLASSES, dtype=np.float16),
                            (128, NUM_CLASSES)))
    wtab = _CACHE["wtab"]
    iot = _CACHE["iot"]
    xf16 = (-np.asarray(logits, np.float32)).astype(np.float16)
    tgt = np.asarray(targets).astype(np.int32)
    in_maps = []
    for core in range(N_CORES):
        sl = slice(core * B_SHARD, (core + 1) * B_SHARD)
        t = tgt[sl]                                    # [8192], sample s
        # wrapped int16 index tile: index j lives at [j%16, j//16],
        # replicated across the 8 gpsimd core blocks
        wrap = np.ascontiguousarray(t.reshape(-1, 16).T.astype(np.int16))
        ix = np.tile(wrap, (8, 1))
        # per-sample (t+180)%360 as f32; sample s = g*128 + p
        t2d = t.reshape(GROUPS, 128).T                 # [p, g]
        t180 = ((t2d + 180) % NUM_CLASSES).astype(np.float32)
        in_maps.append({
            "xf16": np.ascontiguousarray(xf16[sl]),
            "ix": np.ascontiguousarray(ix),
            "t180": np.ascontiguousarray(t180),
            "iot": iot,
            "wtab": wtab,
        })
    return in_maps


def kernel(logits, targets):
    logits = np.asarray(logits, dtype=np.float32)
    targets_np = np.asarray(targets).astype(np.int64)
    assert logits.shape == (B_TOTAL, NUM_CLASSES), logits.shape
    assert targets_np.shape == (B_TOTAL,), targets_np.shape

    nc = _get_nc()
    in_maps = _prep_in_maps(logits, targets_np)
    res = run_bass_kernel_spmd(nc, in_maps, core_ids=list(range(N_CORES)))
    total = np.float64(0.0)
    for out_map in res.results:
        total += np.asarray(out_map["acc"], np.float64).sum()
    loss = np.float32(total / B_TOTAL)
    return (loss, 0.0, loss)
